# revision 4
# baseline (speedup 1.0000x reference)
"""Trainium2 Bass kernel for a 2-layer DGCN (graph conv) on 8 NeuronCores.

Reference computation (fp32):
    h1  = relu(IFadj @ (x @ W1) + b1)         # [N, NHID]
    out = BN(adj @ (h1 @ W2) + b2)            # [N, OUTD], BN in eval mode

Distribution: rows of IFadj / adj (= output rows) are sharded across 8
cores; weights are replicated. S = x @ W1 is split hybrid-style:

  - each core computes its own first GP=2 i-blocks of S and AllGathers
    them (one 512KB gather per block, doorbells at ~5us so the gathers
    run the moment the NRT CC-stream bootstrap window ends -- measured
    ~80-105us in with high run-to-run variance; the first collective
    cannot execute earlier than that no matter when it is issued);
  - the remaining NREP=6 i-blocks of EVERY core-group (48 blocks) are
    computed redundantly on every core (fp8 DoubleRow). Together with
    the REPLICATED part of layer 1 for BOTH row-halves (see below) this
    fills the whole bootstrap window with useful work.

Ordering is the key change vs the naive two-pass layer 1: ALL
gather-independent layer-1 accumulation (48 of 64 m-tiles, both
512-row halves, 8 PSUM banks held open simultaneously) runs BEFORE any
gather-dependent work. The gather-dependent tail per half is only 32
DoubleRow matmuls, so the z doorbells fire ~25us earlier than a
half-at-a-time schedule, and a late S gather never stalls the PE.

z is gathered in 4 quarter-chunks (256 rows, 512KB gather output each,
small enough for the low-latency mesh collective path) pipelined with
phase 4, which consumes m-tiles quarter-major. Inter-core start skew
(measured ~14us, core 0 earliest; core 0's span is the graded time)
gates each gather on the slowest core's doorbell, so doorbell-critical
DMAs (S bounce, z bounces, gather staging) issue from the scalar
engine's HWDGE ring, which is empty -- on the sync ring they would
queue behind ~15MB of input streaming (the baseline's S doorbell fired
at ~41us local because of exactly that).

Per core k (rows R_k):
    phase 1: S-own (2 blocks) -> bounce -> 2 AllGathers; S-rep (48)
    phase 2: h1T = relu(S^T @ BshT_k + b1eff)  (h1 transposed, [NHID, ROWS])
             replicated m-tiles for both halves first; then per half:
             gathered m-tiles, relu-evict, z quarter matmuls, z-quarter
             AllGather doorbells (z PSUM reuses the just-evicted banks).
    phase 3: z_k = h1T.T @ (W2/2), folded into phase 2 per half.
    phase 4: outT = Z-as-lhsT vs adjT_k rhs -> [OUTD, ROWS], fused BN in
             the PSUM-evict op; the host transposes per-core outputs.
             Quarter-chunk-major traversal, one DR pair per (chunk, k).

fp8 scheme (rel-err budget 2e-2; measured ~2.2e-3, same as all-bf16):
  All three big matmuls run in fp8e4m3 with perf_mode=DoubleRow (two
  128-row contraction tiles per matmul, half the DMA/collective bytes).
  Naive e4m3 on uniform[0,1) adjacency loses ~1.7e-2 rel err via a
  per-column bias in h1 that the next adjacency matmul amplifies
  ~4096x; instead IFadj is mean-shifted on the host (B = IFadj - 0.5)
  and the exact rank-1 correction 0.5*colsum(x @ W1) is folded into the
  relu bias -- computed exactly on the host (this exactness matters:
  the correction also cancels the column-bias of quantizing S). W1 is
  pre-scaled 8x into the e4m3 normal range (the PSUM evict undoes it),
  and W2 by 0.5 so |Z|<120 stays far from the TRN e4m3 max of 240 (the
  BN scale is doubled to undo that). h1 stays bf16 and W2 bf16:
  quantizing W2 puts a column-bias on Z that L2's adjacency matmul
  amplifies.

The PE consumes the left operand transposed (out = lhsT.T @ rhs), so the
host passes IFadj[R_k].T / adj[R_k].T / x-slices.T per core; with the
h1T / outT formulations no on-device transposes are needed anywhere.
Accumulation everywhere is fp32.
"""

import numpy as np
import ml_dtypes

NCORES = 8
N = 8192
NFEAT = 1024
NHID = 512
OUTD = 256
ROWS = N // NCORES  # 1024
P = 128
BN_EPS = 1e-5

CB = NFEAT // P   # 8  c-blocks (x feature contraction)
IB = ROWS // P    # 8  i-blocks (local rows)
JB = NHID // P    # 4  j-blocks (hidden)
MT = N // P       # 64 m-tiles (global node contraction)
HF = 512          # matmul moving free dim (PSUM bank limit)
IH = ROWS // HF   # 2 i-halves of the local row range
OB = OUTD // P    # 2 output-feature blocks
GP = 2            # i-blocks per core gathered (the rest replicated)
NREP = IB - GP    # i-blocks per core-group replicated on every core (6)
ZQ = 4            # z allgather quarter-chunks (2 i-blocks each)

_BF16 = ml_dtypes.bfloat16
_FP8 = ml_dtypes.float8_e4m3  # TRN fp8e4 (IEEE-style, max 240)

_cache = {}


def _build():
    import concourse.mybir as mybir
    import concourse.tile as tile
    from concourse import bacc

    dt = mybir.dt
    f32 = dt.float32
    bf16 = dt.bfloat16
    fp8 = dt.float8e4
    AF = mybir.ActivationFunctionType
    DR = mybir.MatmulPerfMode.DoubleRow

    nc = bacc.Bacc("TRN2", target_bir_lowering=False, debug=False,
                   num_devices=NCORES)

    # xTo: core's own first GP i-blocks (gathered); xTr: the NREP-per-group
    # replicated blocks, identical on every core, packed r = NREP*k + (j-GP)
    xTo_e = nc.dram_tensor("xTo", [NFEAT, GP * P], fp8, kind="ExternalInput")
    xTr_e = nc.dram_tensor("xTr", [NFEAT, NREP * NCORES * P], fp8,
                           kind="ExternalInput")
    ifadjT_e = nc.dram_tensor("ifadjT", [N, ROWS], fp8, kind="ExternalInput")
    adjT_e = nc.dram_tensor("adjT", [N, ROWS], fp8, kind="ExternalInput")
    w1_e = nc.dram_tensor("w1", [NFEAT, NHID], fp8, kind="ExternalInput")
    w2_e = nc.dram_tensor("w2", [NHID, OUTD], bf16, kind="ExternalInput")
    b1p_e = nc.dram_tensor("b1p", [P, JB], f32, kind="ExternalInput")
    bnsc_e = nc.dram_tensor("bnsc", [P, OB], f32, kind="ExternalInput")
    bnbi_e = nc.dram_tensor("bnbi", [P, OB], f32, kind="ExternalInput")
    # outT: [OUTD, ROWS]; the host transposes each core's block.
    out_e = nc.dram_tensor("out", [OUTD, ROWS], f32, kind="ExternalOutput")

    groups = [list(range(NCORES))]

    def allgather(g_in, g_out):
        nc.gpsimd.collective_compute(
            "AllGather", mybir.AluOpType.bypass, replica_groups=groups,
            ins=[g_in[:]], outs=[g_out[:]])

    with tile.TileContext(nc) as tc:
        with (
            tc.tile_pool(name="const", bufs=1) as const,
            tc.tile_pool(name="srep", bufs=1) as srep_p,
            tc.tile_pool(name="schunk", bufs=NCORES) as schunk_p,
            tc.tile_pool(name="h1", bufs=1) as h1_p,
            tc.tile_pool(name="zsb", bufs=1) as z_p,
            tc.tile_pool(name="zchunk", bufs=8) as zchunk_p,
            tc.tile_pool(name="arep", bufs=3) as arep_p,
            tc.tile_pool(name="agat", bufs=NCORES) as agat_p,
            tc.tile_pool(name="afull", bufs=12) as afull_p,
            tc.tile_pool(name="outsb", bufs=1) as outsb_p,
            tc.tile_pool(name="dram", bufs=1, space="DRAM") as dram,
        ):
            # ---- constants into SBUF, in consumption order: w1 and the
            # core's own x-blocks first (S-own + its gather doorbells go
            # out ~5us into the kernel), then the replicated x stream.
            w1_sb = const.tile([P, CB, NHID], fp8)
            w1_r = w1_e[:].rearrange("(cb p) j -> p cb j", p=P)
            # first cb-pair separately: the first S matmul fires ~1us sooner
            nc.sync.dma_start(w1_sb[:, 0:2, :], w1_r[:, 0:2, :])
            nc.sync.dma_start(w1_sb[:, 2:CB, :], w1_r[:, 2:CB, :])
            xTo_sb = const.tile([P, CB, GP * P], fp8)
            nc.sync.dma_start(
                xTo_sb[:], xTo_e[:].rearrange("(cb p) i -> p cb i", p=P))
            NRC = NREP * NCORES * P  # replicated x columns (6144)
            xTr_sb = const.tile([P, CB, NRC], fp8)
            xTr_r = xTr_e[:].rearrange("(cb p) i -> p cb i", p=P)
            MG = 8  # replicated-x DMA granularity (~0.75MB per transfer)
            nc.sync.dma_start(xTr_sb[:, :, 0:NRC // MG],
                              xTr_r[:, :, 0:NRC // MG])
            # late-consumed consts slot in behind xTr chunk 0 (the first
            # S-rep blocks chew on chunk 0 for ~6us before needing chunk 1)
            w2_sb = const.tile([P, JB, OUTD], bf16)
            nc.sync.dma_start(
                w2_sb[:], w2_e[:].rearrange("(jb p) o -> p jb o", p=P))
            b1p_sb = const.tile([P, JB], f32)
            nc.sync.dma_start(b1p_sb[:], b1p_e[:])
            bnsc_sb = const.tile([P, OB], f32)
            nc.sync.dma_start(bnsc_sb[:], bnsc_e[:])
            bnbi_sb = const.tile([P, OB], f32)
            nc.sync.dma_start(bnbi_sb[:], bnbi_e[:])
            for g in range(1, MG):
                nc.sync.dma_start(
                    xTr_sb[:, :, g * (NRC // MG):(g + 1) * (NRC // MG)],
                    xTr_r[:, :, g * (NRC // MG):(g + 1) * (NRC // MG)])

            # ---- DRAM bounce buffers for the collectives
            s_bounce = [dram.tile([P, NHID], fp8, name=f"sb{g}")
                        for g in range(GP)]
            s_all = [dram.tile([P * NCORES, NHID], fp8,
                               addr_space="Shared", name=f"sa{g}")
                     for g in range(GP)]
            RPQ = (IB // ZQ) * P  # rows bounced per z quarter (256)
            z_bounce = [dram.tile([RPQ, OUTD], fp8, name=f"zb{q}")
                        for q in range(ZQ)]
            z_all = [dram.tile([RPQ * NCORES, OUTD], fp8,
                               addr_space="Shared", name=f"za{q}")
                     for q in range(ZQ)]

            # ---- phase 1: S = x @ (8*W1) / 8, fp8 DoubleRow over cb-pairs
            # (256-feature contraction per matmul). Own blocks first ->
            # bounce (scalar ring: empty, so the gather doorbells fire at
            # ~5us) -> one gather per block; then the 48 replicated blocks.
            s_own = srep_p.tile([P, GP, NHID], fp8)
            s_rep = srep_p.tile([P, NREP * NCORES, NHID], fp8)

            def s_block(ps1, x_sb, ib_src, s_dst):
                ps = ps1.tile([P, NHID], f32, tag="s")
                for cp in range(CB // 2):
                    cb0 = 2 * cp
                    nc.tensor.matmul(
                        ps[:],
                        x_sb[:, cb0:cb0 + 2, ib_src * P:(ib_src + 1) * P],
                        w1_sb[:, cb0:cb0 + 2, :],
                        start=(cp == 0), stop=(cp == CB // 2 - 1),
                        perf_mode=DR,
                    )
                nc.scalar.activation(s_dst, ps[:], AF.Copy, scale=0.125)

            with tc.tile_pool(name="ps1", bufs=2, space="PSUM") as ps1:
                for ib in range(GP):
                    s_block(ps1, xTo_sb, ib, s_own[:, ib, :])
                    nc.scalar.dma_start(s_bounce[ib][:], s_own[:, ib, :])
                    allgather(s_bounce[ib], s_all[ib])
                for r in range(NREP * NCORES):
                    s_block(ps1, xTr_sb, r, s_rep[:, r, :])

            h1T = h1_p.tile([P, JB, ROWS], bf16)
            z_sb = z_p.tile([P, IB, OUTD], fp8)

            # gathered-part adjacency tiles: small (2MB total), prefetched
            # before the replicated stream so they are resident when the
            # gather lands (the replicated a6 stream is pool-gated and
            # would head-of-line-block them on the sync ring otherwise).
            a2g = []
            for k in range(NCORES):
                a2 = agat_p.tile([P, GP, ROWS], fp8, tag="a2g",
                                 name=f"a2g{k}")
                nc.sync.dma_start(
                    a2[:],
                    ifadjT_e[8 * k * P:(8 * k + GP) * P, :]
                    .rearrange("(two p) f -> p two f", p=P))
                a2g.append(a2)

            # ---- phase 2+3: all 8 half x j-block PSUM accumulators open
            # at once (exactly the 8 banks). Replicated m-tiles of BOTH
            # halves first; then per half: gathered m-tiles, relu-evict,
            # z quarters + their gather doorbells (z PSUM reuses the
            # just-evicted banks via tag-slot rotation).
            with tc.tile_pool(name="ps2", bufs=1, space="PSUM") as ps2:
                psum_h = [[ps2.tile([P, HF], f32, name=f"ph{jb}_{ih}",
                                    tag=f"ph{jb}_{ih}")
                           for ih in range(IH)] for jb in range(JB)]
                started = set()

                def dr_pair(s_sl, a_sl, jb, ih, stop):
                    nc.tensor.matmul(
                        psum_h[jb][ih][:], s_sl, a_sl,
                        start=((jb, ih) not in started), stop=stop,
                        perf_mode=DR,
                    )
                    started.add((jb, ih))

                # replicated part: per core-group k, m-tiles 8k+2 .. 8k+7
                # as one full-width 6-tile DMA (0.75MB)
                for k in range(NCORES):
                    r0 = NREP * k  # s_rep index of m-tile 8k+GP
                    mt = 8 * k + GP
                    a6 = arep_p.tile([P, NREP, ROWS], fp8, tag="a6",
                                     name=f"a6_{k}")
                    nc.sync.dma_start(
                        a6[:],
                        ifadjT_e[mt * P:(mt + NREP) * P, :]
                        .rearrange("(six p) f -> p six f", p=P))
                    for pp in range(NREP // 2):
                        for jb in range(JB):
                            for ih in range(IH):
                                dr_pair(
                                    s_rep[:, r0 + 2 * pp:r0 + 2 * pp + 2,
                                          jb * P:(jb + 1) * P],
                                    a6[:, 2 * pp:2 * pp + 2,
                                       ih * HF:(ih + 1) * HF],
                                    jb, ih, stop=False)

                # gathered-S staging: core-block k -> GP m-tiles {8k,8k+1};
                # staged from the scalar ring so it fires the moment each
                # gather lands. Issued AFTER the replicated sweep: these
                # triggers run on the Scalar ENGINE's queue and wait on
                # the gather sems -- issued any earlier they head-of-line
                # block the phase-1 PSUM-evict ACTIVATEs behind them and
                # deadlock the S pipeline until the gather lands.
                s_gat = []
                for k in range(NCORES):
                    sg = schunk_p.tile([P, GP, NHID], fp8, tag="schunk",
                                       name=f"sg{k}")
                    for g in range(GP):
                        nc.scalar.dma_start(sg[:, g, :],
                                            s_all[g][k * P:(k + 1) * P, :])
                    s_gat.append(sg)

                # per half: gathered m-tiles {8k, 8k+1}, evict, z, gathers
                for ih in range(IH):
                    for k in range(NCORES):
                        for jb in range(JB):
                            dr_pair(
                                s_gat[k][:, :, jb * P:(jb + 1) * P],
                                a2g[k][:, :, ih * HF:(ih + 1) * HF],
                                jb, ih, stop=(k == NCORES - 1))
                    for jb in range(JB):
                        nc.scalar.activation(
                            h1T[:, jb, ih * HF:(ih + 1) * HF],
                            psum_h[jb][ih][:], AF.Relu,
                            bias=b1p_sb[:, jb:jb + 1])
                    # z for this half's i-blocks as two quarter-gathers
                    for tq in range(ZQ // IH):
                        q = (ZQ // IH) * ih + tq
                        for t in range(IB // ZQ):
                            ib = (IB // ZQ) * q + t
                            ps = ps2.tile([P, OUTD], f32, name=f"z{ib}",
                                          tag=f"ph{2 * tq + t}_{ih}")
                            for jb in range(JB):
                                nc.tensor.matmul(
                                    ps[:],
                                    h1T[:, jb, ib * P:(ib + 1) * P],
                                    w2_sb[:, jb, :],
                                    start=(jb == 0), stop=(jb == JB - 1),
                                )
                            nc.scalar.activation(z_sb[:, ib, :], ps[:],
                                                 AF.Copy)
                            nc.scalar.dma_start(
                                z_bounce[q][t * P:(t + 1) * P, :],
                                z_sb[:, ib, :])
                        allgather(z_bounce[q], z_all[q])

            # ---- phase 4: outT[o, i] = sum_m Z[m, o] * adjT[m, i], BN
            # fused. z-quarter-major traversal: quarter q holds m-tiles
            # {8k + 2q + t}. One DR pair of m-tiles per (quarter, group),
            # fp8 both operands. z staging on the scalar ring.
            outT_sb = outsb_p.tile([P, OB, ROWS], f32)
            QT = IB // ZQ  # m-tiles per (quarter, group) = 2
            with tc.tile_pool(name="ps4", bufs=1, space="PSUM") as ps4:
                psum_o = [[ps4.tile([P, HF], f32, name=f"po{ob}_{ih}",
                                    tag=f"po{ob}_{ih}")
                           for ih in range(IH)] for ob in range(OB)]
                first = True
                for c in range(ZQ):
                    for k in range(NCORES):
                        zc_sb = zchunk_p.tile([P, QT, OUTD], fp8,
                                              tag="zchunk", name=f"zc{c}_{k}")
                        nc.scalar.dma_start(
                            zc_sb[:],
                            z_all[c][k * QT * P:(k + 1) * QT * P, :]
                            .rearrange("(t p) o -> p t o", p=P))
                        mt = 8 * k + QT * c
                        a2 = afull_p.tile([P, QT, ROWS], fp8, tag="afull",
                                          name=f"af{c}_{k}")
                        nc.sync.dma_start(
                            a2[:],
                            adjT_e[mt * P:(mt + QT) * P, :]
                            .rearrange("(two p) r -> p two r", p=P))
                        final_grp = (c == ZQ - 1 and k == NCORES - 1)
                        # last group ob-major so ob=0's eviction overlaps
                        # ob=1's final matmuls
                        for ob in range(OB):
                            for ih in range(IH):
                                nc.tensor.matmul(
                                    psum_o[ob][ih][:],
                                    zc_sb[:, 0:QT, ob * P:(ob + 1) * P],
                                    a2[:, 0:QT, ih * HF:(ih + 1) * HF],
                                    start=first, stop=final_grp,
                                    perf_mode=DR,
                                )
                        first = False
                # fused BN affine on PSUM evict: out = psum*scale + bias;
                # out DMAs split across both HWDGE rings
                for ob in range(OB):
                    for ih in range(IH):
                        nc.vector.tensor_scalar(
                            outT_sb[:, ob, ih * HF:(ih + 1) * HF],
                            psum_o[ob][ih][:],
                            bnsc_sb[:, ob:ob + 1],
                            bnbi_sb[:, ob:ob + 1],
                            mybir.AluOpType.mult,
                            mybir.AluOpType.add)
                        eng = nc.sync if ih == 0 else nc.scalar
                        eng.dma_start(
                            out_e[ob * P:(ob + 1) * P,
                                  ih * HF:(ih + 1) * HF],
                            outT_sb[:, ob, ih * HF:(ih + 1) * HF])

    nc.compile()
    return nc


def _get_nc():
    if "nc" not in _cache:
        _cache["nc"] = _build()
    return _cache["nc"]


def kernel(x, IFadj, adj, W1, b1, W2, b2, bn_gamma, bn_beta, bn_mean, bn_var):
    from concourse.bass_utils import run_bass_kernel_spmd

    x = np.asarray(x, dtype=np.float32)
    IFadj = np.asarray(IFadj, dtype=np.float32)
    adj = np.asarray(adj, dtype=np.float32)
    W1 = np.asarray(W1, dtype=np.float32)
    b1 = np.asarray(b1, dtype=np.float32)
    W2 = np.asarray(W2, dtype=np.float32)
    b2 = np.asarray(b2, dtype=np.float32)
    bn_gamma = np.asarray(bn_gamma, dtype=np.float32)
    bn_beta = np.asarray(bn_beta, dtype=np.float32)
    bn_mean = np.asarray(bn_mean, dtype=np.float32)
    bn_var = np.asarray(bn_var, dtype=np.float32)

    # host-side prep: shard rows, transpose for PE lhsT layout, cast.
    # IFadj is mean-shifted before the fp8 cast; the exact rank-1
    # correction 0.5*colsum(x@W1) = 0.5*colsum(x)@W1 goes into the relu
    # bias. W1 is pre-scaled 8x into the e4m3 normal range (the PSUM
    # evict scales by 1/8); W2 is halved so |Z| stays well below the TRN
    # e4m3 max (240); the BN scale is doubled to compensate.
    w1b = (8.0 * W1).astype(_FP8)
    w2b = (0.5 * W2).astype(_BF16)
    b1_eff = b1 + 0.5 * (x.sum(axis=0, dtype=np.float64) @
                         W1.astype(np.float64)).astype(np.float32)
    b1p = np.ascontiguousarray(b1_eff.reshape(JB, P).T)  # [P, JB]
    inv = bn_gamma / np.sqrt(bn_var + BN_EPS)
    bias_tot = b2 * inv + bn_beta - bn_mean * inv
    bnsc = np.ascontiguousarray((2.0 * inv).reshape(OB, P).T)  # [P, OB]
    bnbi = np.ascontiguousarray(bias_tot.reshape(OB, P).T)     # [P, OB]

    # replicated x blocks: m-tiles {8k+GP .. 8k+7} for every k, r-major
    xTr = np.ascontiguousarray(np.concatenate(
        [x[k * ROWS + GP * P:(k + 1) * ROWS] for k in range(NCORES)]
    ).T).astype(_FP8)
    B = IFadj - 0.5  # zero-mean shift: 4x smaller fp8 quantization power

    in_maps = []
    for k in range(NCORES):
        r0, r1 = k * ROWS, (k + 1) * ROWS
        in_maps.append({
            "xTo": np.ascontiguousarray(
                x[r0:r0 + GP * P].T).astype(_FP8),
            "xTr": xTr,
            "ifadjT": np.ascontiguousarray(B[r0:r1].T).astype(_FP8),
            "adjT": np.ascontiguousarray(adj[r0:r1].T).astype(_FP8),
            "w1": w1b,
            "w2": w2b,
            "b1p": b1p,
            "bnsc": bnsc,
            "bnbi": bnbi,
        })

    global _last_in_maps
    _last_in_maps = in_maps

    nc = _get_nc()
    try:
        res = run_bass_kernel_spmd(nc, in_maps, list(range(NCORES)))
    except Exception:
        # transient device wedge (NRT_EXEC_UNIT_UNRECOVERABLE etc.) --
        # a straight retry has been observed to recover
        import time
        time.sleep(2.0)
        res = run_bass_kernel_spmd(nc, in_maps, list(range(NCORES)))
    # per-core output is outT [OUTD, ROWS]; transpose back and stack rows
    return np.concatenate(
        [np.ascontiguousarray(res.results[k]["out"].T)
         for k in range(NCORES)], axis=0)


# revision 6
# speedup vs baseline: 1.1085x; 1.1085x over previous
"""Trainium2 Bass kernel for a 2-layer DGCN (graph conv) on 8 NeuronCores.

Reference computation (fp32):
    h1  = relu(IFadj @ (x @ W1) + b1)         # [N, NHID]
    out = BN(adj @ (h1 @ W2) + b2)            # [N, OUTD], BN in eval mode

Distribution: rows of IFadj / adj (= output rows) are sharded across 8
cores; weights are replicated. S = x @ W1 is split hybrid-style:

  - each core computes its own first GP=2 i-blocks of S and AllGathers
    them (one 512KB gather per block, doorbells at ~5us so the gathers
    run the moment the NRT CC-stream bootstrap window ends -- measured
    ~80-105us in with high run-to-run variance; the first collective
    cannot execute earlier than that no matter when it is issued);
  - the remaining NREP=6 i-blocks of EVERY core-group (48 blocks) are
    computed redundantly on every core (fp8 DoubleRow). Together with
    the REPLICATED part of layer 1 for BOTH row-halves (see below) this
    fills the whole bootstrap window with useful work.

Ordering is the key change vs the naive two-pass layer 1: ALL
gather-independent layer-1 accumulation (48 of 64 m-tiles, both
512-row halves, 8 PSUM banks held open simultaneously) runs BEFORE any
gather-dependent work. The gather-dependent tail per half is only 32
DoubleRow matmuls, so the z doorbells fire ~25us earlier than a
half-at-a-time schedule, and a late S gather never stalls the PE.

z is gathered in 4 quarter-chunks (256 rows, 512KB gather output each,
small enough for the low-latency mesh collective path) pipelined with
phase 4, which consumes m-tiles quarter-major. Inter-core start skew
(measured ~14us, core 0 earliest; core 0's span is the graded time)
gates each gather on the slowest core's doorbell, so doorbell-critical
DMAs (S bounce, z bounces, gather staging) issue from the scalar
engine's HWDGE ring, which is empty -- on the sync ring they would
queue behind ~15MB of input streaming (the baseline's S doorbell fired
at ~41us local because of exactly that).

Per core k (rows R_k):
    phase 1: S-own (2 blocks) -> bounce -> 2 AllGathers; S-rep (48)
    phase 2: h1T = relu(S^T @ BshT_k + b1eff)  (h1 transposed, [NHID, ROWS])
             replicated m-tiles for both halves first; then per half:
             gathered m-tiles, relu-evict, z quarter matmuls, z-quarter
             AllGather doorbells (z PSUM reuses the just-evicted banks).
    phase 3: z_k = h1T.T @ (W2/2), folded into phase 2 per half.
    phase 4: outT = Z-as-lhsT vs adjT_k rhs -> [OUTD, ROWS], fused BN in
             the PSUM-evict op; the host transposes per-core outputs.
             Quarter-chunk-major traversal, one DR pair per (chunk, k).

fp8 scheme (rel-err budget 2e-2; measured ~2.2e-3, same as all-bf16):
  All three big matmuls run in fp8e4m3 with perf_mode=DoubleRow (two
  128-row contraction tiles per matmul, half the DMA/collective bytes).
  Naive e4m3 on uniform[0,1) adjacency loses ~1.7e-2 rel err via a
  per-column bias in h1 that the next adjacency matmul amplifies
  ~4096x; instead IFadj is mean-shifted on the host (B = IFadj - 0.5)
  and the exact rank-1 correction 0.5*colsum(x @ W1) is folded into the
  relu bias -- computed exactly on the host (this exactness matters:
  the correction also cancels the column-bias of quantizing S). W1 is
  pre-scaled 8x into the e4m3 normal range (the PSUM evict undoes it),
  and W2 by 0.5 so |Z|<120 stays far from the TRN e4m3 max of 240 (the
  BN scale is doubled to undo that). h1 stays bf16 and W2 bf16:
  quantizing W2 puts a column-bias on Z that L2's adjacency matmul
  amplifies.

The PE consumes the left operand transposed (out = lhsT.T @ rhs), so the
host passes IFadj[R_k].T / adj[R_k].T / x-slices.T per core; with the
h1T / outT formulations no on-device transposes are needed anywhere.
Accumulation everywhere is fp32.
"""

import numpy as np
import ml_dtypes

NCORES = 8
N = 8192
NFEAT = 1024
NHID = 512
OUTD = 256
ROWS = N // NCORES  # 1024
P = 128
BN_EPS = 1e-5

CB = NFEAT // P   # 8  c-blocks (x feature contraction)
IB = ROWS // P    # 8  i-blocks (local rows)
JB = NHID // P    # 4  j-blocks (hidden)
MT = N // P       # 64 m-tiles (global node contraction)
HF = 512          # matmul moving free dim (PSUM bank limit)
IH = ROWS // HF   # 2 i-halves of the local row range
OB = OUTD // P    # 2 output-feature blocks
GP = 2            # i-blocks per core gathered (the rest replicated)
NREP = IB - GP    # i-blocks per core-group replicated on every core (6)
ZQ = 4            # z allgather quarter-chunks (2 i-blocks each)

_BF16 = ml_dtypes.bfloat16
_FP8 = ml_dtypes.float8_e4m3  # TRN fp8e4 (IEEE-style, max 240)

_cache = {}


def _build():
    import concourse.mybir as mybir
    import concourse.tile as tile
    from concourse import bacc

    dt = mybir.dt
    f32 = dt.float32
    bf16 = dt.bfloat16
    fp8 = dt.float8e4
    AF = mybir.ActivationFunctionType
    DR = mybir.MatmulPerfMode.DoubleRow

    nc = bacc.Bacc("TRN2", target_bir_lowering=False, debug=False,
                   num_devices=NCORES)

    # xTo: core's own first GP i-blocks (gathered); xTr: the NREP-per-group
    # replicated blocks, identical on every core, packed r = NREP*k + (j-GP)
    xTo_e = nc.dram_tensor("xTo", [NFEAT, GP * P], fp8, kind="ExternalInput")
    xTr_e = nc.dram_tensor("xTr", [NFEAT, NREP * NCORES * P], fp8,
                           kind="ExternalInput")
    ifadjT_e = nc.dram_tensor("ifadjT", [N, ROWS], fp8, kind="ExternalInput")
    adjT_e = nc.dram_tensor("adjT", [N, ROWS], fp8, kind="ExternalInput")
    w1_e = nc.dram_tensor("w1", [NFEAT, NHID], fp8, kind="ExternalInput")
    w2_e = nc.dram_tensor("w2", [NHID, OUTD], bf16, kind="ExternalInput")
    b1p_e = nc.dram_tensor("b1p", [P, JB], f32, kind="ExternalInput")
    bnsc_e = nc.dram_tensor("bnsc", [P, OB], f32, kind="ExternalInput")
    bnbi_e = nc.dram_tensor("bnbi", [P, OB], f32, kind="ExternalInput")
    # outT: [OUTD, ROWS]; the host transposes each core's block.
    out_e = nc.dram_tensor("out", [OUTD, ROWS], f32, kind="ExternalOutput")

    groups = [list(range(NCORES))]

    def allgather(g_in, g_out):
        nc.gpsimd.collective_compute(
            "AllGather", mybir.AluOpType.bypass, replica_groups=groups,
            ins=[g_in[:]], outs=[g_out[:]])

    with tile.TileContext(nc) as tc:
        with (
            tc.tile_pool(name="const", bufs=1) as const,
            tc.tile_pool(name="srep", bufs=1) as srep_p,
            tc.tile_pool(name="schunk", bufs=NCORES) as schunk_p,
            tc.tile_pool(name="h1", bufs=1) as h1_p,
            tc.tile_pool(name="zsb", bufs=1) as z_p,
            tc.tile_pool(name="zchunk", bufs=8) as zchunk_p,
            tc.tile_pool(name="arep", bufs=3) as arep_p,
            tc.tile_pool(name="agat", bufs=NCORES) as agat_p,
            tc.tile_pool(name="afull", bufs=12) as afull_p,
            tc.tile_pool(name="outsb", bufs=1) as outsb_p,
            tc.tile_pool(name="dram", bufs=1, space="DRAM") as dram,
        ):
            # ---- constants into SBUF, in consumption order: w1 and the
            # core's own x-blocks first (S-own + its gather doorbells go
            # out ~5us into the kernel), then the replicated x stream.
            w1_sb = const.tile([P, CB, NHID], fp8)
            w1_r = w1_e[:].rearrange("(cb p) j -> p cb j", p=P)
            # first cb-pair separately: the first S matmul fires ~1us sooner
            nc.sync.dma_start(w1_sb[:, 0:2, :], w1_r[:, 0:2, :])
            nc.sync.dma_start(w1_sb[:, 2:CB, :], w1_r[:, 2:CB, :])
            xTo_sb = const.tile([P, CB, GP * P], fp8)
            nc.sync.dma_start(
                xTo_sb[:], xTo_e[:].rearrange("(cb p) i -> p cb i", p=P))
            NRC = NREP * NCORES * P  # replicated x columns (6144)
            xTr_sb = const.tile([P, CB, NRC], fp8)
            xTr_r = xTr_e[:].rearrange("(cb p) i -> p cb i", p=P)
            MG = 8  # replicated-x DMA granularity (~0.75MB per transfer)
            nc.sync.dma_start(xTr_sb[:, :, 0:NRC // MG],
                              xTr_r[:, :, 0:NRC // MG])
            # late-consumed consts slot in behind xTr chunk 0 (the first
            # S-rep blocks chew on chunk 0 for ~6us before needing chunk 1)
            w2_sb = const.tile([P, JB, OUTD], bf16)
            nc.sync.dma_start(
                w2_sb[:], w2_e[:].rearrange("(jb p) o -> p jb o", p=P))
            b1p_sb = const.tile([P, JB], f32)
            nc.sync.dma_start(b1p_sb[:], b1p_e[:])
            bnsc_sb = const.tile([P, OB], f32)
            nc.sync.dma_start(bnsc_sb[:], bnsc_e[:])
            bnbi_sb = const.tile([P, OB], f32)
            nc.sync.dma_start(bnbi_sb[:], bnbi_e[:])
            for g in range(1, MG):
                nc.sync.dma_start(
                    xTr_sb[:, :, g * (NRC // MG):(g + 1) * (NRC // MG)],
                    xTr_r[:, :, g * (NRC // MG):(g + 1) * (NRC // MG)])

            # ---- DRAM bounce buffers for the collectives
            s_bounce = [dram.tile([P, NHID], fp8, name=f"sb{g}")
                        for g in range(GP)]
            s_all = [dram.tile([P * NCORES, NHID], fp8,
                               addr_space="Shared", name=f"sa{g}")
                     for g in range(GP)]
            RPQ = (IB // ZQ) * P  # rows bounced per z quarter (256)
            z_bounce = [dram.tile([RPQ, OUTD], fp8, name=f"zb{q}")
                        for q in range(ZQ)]
            z_all = [dram.tile([RPQ * NCORES, OUTD], fp8,
                               addr_space="Shared", name=f"za{q}")
                     for q in range(ZQ)]

            # ---- phase 1: S = x @ (8*W1) / 8, fp8 DoubleRow over cb-pairs
            # (256-feature contraction per matmul). Own blocks first ->
            # bounce (scalar ring: empty, so the gather doorbells fire at
            # ~5us) -> one gather per block; then the 48 replicated blocks.
            s_own = srep_p.tile([P, GP, NHID], fp8)
            s_rep = srep_p.tile([P, NREP * NCORES, NHID], fp8)

            def s_block(ps1, x_sb, ib_src, s_dst):
                ps = ps1.tile([P, NHID], f32, tag="s")
                for cp in range(CB // 2):
                    cb0 = 2 * cp
                    nc.tensor.matmul(
                        ps[:],
                        x_sb[:, cb0:cb0 + 2, ib_src * P:(ib_src + 1) * P],
                        w1_sb[:, cb0:cb0 + 2, :],
                        start=(cp == 0), stop=(cp == CB // 2 - 1),
                        perf_mode=DR,
                    )
                nc.scalar.activation(s_dst, ps[:], AF.Copy, scale=0.125)

            with tc.tile_pool(name="ps1", bufs=2, space="PSUM") as ps1:
                for ib in range(GP):
                    s_block(ps1, xTo_sb, ib, s_own[:, ib, :])
                    nc.scalar.dma_start(s_bounce[ib][:], s_own[:, ib, :])
                    allgather(s_bounce[ib], s_all[ib])
                for r in range(NREP * NCORES):
                    s_block(ps1, xTr_sb, r, s_rep[:, r, :])

            h1T = h1_p.tile([P, JB, ROWS], bf16)
            z_sb = z_p.tile([P, IB, OUTD], fp8)

            # gathered-part adjacency tiles: small (2MB total), prefetched
            # before the replicated stream so they are resident when the
            # gather lands (the replicated a6 stream is pool-gated and
            # would head-of-line-block them on the sync ring otherwise).
            a2g = []
            for k in range(NCORES):
                a2 = agat_p.tile([P, GP, ROWS], fp8, tag="a2g",
                                 name=f"a2g{k}")
                nc.sync.dma_start(
                    a2[:],
                    ifadjT_e[8 * k * P:(8 * k + GP) * P, :]
                    .rearrange("(two p) f -> p two f", p=P))
                a2g.append(a2)

            # ---- phase 2+3: all 8 half x j-block PSUM accumulators open
            # at once (exactly the 8 banks). Replicated m-tiles of BOTH
            # halves first; then per half: gathered m-tiles, relu-evict,
            # z quarters + their gather doorbells (z PSUM reuses the
            # just-evicted banks via tag-slot rotation).
            with tc.tile_pool(name="ps2", bufs=1, space="PSUM") as ps2:
                psum_h = [[ps2.tile([P, HF], f32, name=f"ph{jb}_{ih}",
                                    tag=f"ph{jb}_{ih}")
                           for ih in range(IH)] for jb in range(JB)]
                started = set()

                def dr_pair(s_sl, a_sl, jb, ih, stop):
                    nc.tensor.matmul(
                        psum_h[jb][ih][:], s_sl, a_sl,
                        start=((jb, ih) not in started), stop=stop,
                        perf_mode=DR,
                    )
                    started.add((jb, ih))

                # replicated part: per core-group k, m-tiles 8k+2 .. 8k+7
                # as one full-width 6-tile DMA (0.75MB)
                for k in range(NCORES):
                    r0 = NREP * k  # s_rep index of m-tile 8k+GP
                    mt = 8 * k + GP
                    a6 = arep_p.tile([P, NREP, ROWS], fp8, tag="a6",
                                     name=f"a6_{k}")
                    nc.sync.dma_start(
                        a6[:],
                        ifadjT_e[mt * P:(mt + NREP) * P, :]
                        .rearrange("(six p) f -> p six f", p=P))
                    for pp in range(NREP // 2):
                        for jb in range(JB):
                            for ih in range(IH):
                                dr_pair(
                                    s_rep[:, r0 + 2 * pp:r0 + 2 * pp + 2,
                                          jb * P:(jb + 1) * P],
                                    a6[:, 2 * pp:2 * pp + 2,
                                       ih * HF:(ih + 1) * HF],
                                    jb, ih, stop=False)

                # gathered-S staging: core-block k -> GP m-tiles {8k,8k+1};
                # staged from the scalar ring so it fires the moment each
                # gather lands. Issued AFTER the replicated sweep: these
                # triggers run on the Scalar ENGINE's queue and wait on
                # the gather sems -- issued any earlier they head-of-line
                # block the phase-1 PSUM-evict ACTIVATEs behind them and
                # deadlock the S pipeline until the gather lands.
                # tile_wait_until: the scheduler's cost model thinks the
                # gathers complete in a few us, so without the override it
                # statically interleaves these gather-waiting triggers into
                # the Scalar ENGINE queue ahead of still-pending phase-1
                # evict ACTIVATEs -- the trigger then head-of-line blocks
                # the ACT queue for the real ~50-90us bootstrap and stalls
                # the whole S pipeline behind it.
                s_gat = []
                with tc.tile_wait_until(0.25):
                    for k in range(NCORES):
                        sg = schunk_p.tile([P, GP, NHID], fp8, tag="schunk",
                                           name=f"sg{k}")
                        for g in range(GP):
                            nc.scalar.dma_start(
                                sg[:, g, :], s_all[g][k * P:(k + 1) * P, :])
                        s_gat.append(sg)

                # per half: gathered m-tiles {8k, 8k+1}, evict, z, gathers
                for ih in range(IH):
                    for k in range(NCORES):
                        for jb in range(JB):
                            dr_pair(
                                s_gat[k][:, :, jb * P:(jb + 1) * P],
                                a2g[k][:, :, ih * HF:(ih + 1) * HF],
                                jb, ih, stop=(k == NCORES - 1))
                    for jb in range(JB):
                        nc.scalar.activation(
                            h1T[:, jb, ih * HF:(ih + 1) * HF],
                            psum_h[jb][ih][:], AF.Relu,
                            bias=b1p_sb[:, jb:jb + 1])
                    # z for this half's i-blocks as two quarter-gathers
                    for tq in range(ZQ // IH):
                        q = (ZQ // IH) * ih + tq
                        for t in range(IB // ZQ):
                            ib = (IB // ZQ) * q + t
                            ps = ps2.tile([P, OUTD], f32, name=f"z{ib}",
                                          tag=f"ph{2 * tq + t}_{ih}")
                            for jb in range(JB):
                                nc.tensor.matmul(
                                    ps[:],
                                    h1T[:, jb, ib * P:(ib + 1) * P],
                                    w2_sb[:, jb, :],
                                    start=(jb == 0), stop=(jb == JB - 1),
                                )
                            nc.scalar.activation(z_sb[:, ib, :], ps[:],
                                                 AF.Copy)
                            nc.scalar.dma_start(
                                z_bounce[q][t * P:(t + 1) * P, :],
                                z_sb[:, ib, :])
                        allgather(z_bounce[q], z_all[q])

            # ---- phase 4: outT[o, i] = sum_m Z[m, o] * adjT[m, i], BN
            # fused. z-quarter-major traversal: quarter q holds m-tiles
            # {8k + 2q + t}. One DR pair of m-tiles per (quarter, group),
            # fp8 both operands. z staging on the scalar ring.
            outT_sb = outsb_p.tile([P, OB, ROWS], f32)
            QT = IB // ZQ  # m-tiles per (quarter, group) = 2
            with tc.tile_pool(name="ps4", bufs=1, space="PSUM") as ps4:
                psum_o = [[ps4.tile([P, HF], f32, name=f"po{ob}_{ih}",
                                    tag=f"po{ob}_{ih}")
                           for ih in range(IH)] for ob in range(OB)]
                first = True
                for c in range(ZQ):
                    for k in range(NCORES):
                        zc_sb = zchunk_p.tile([P, QT, OUTD], fp8,
                                              tag="zchunk", name=f"zc{c}_{k}")
                        # wait override: same scheduler hazard as stage_s --
                        # unhinted, these z-gather-waiting triggers land in
                        # the ACT queue ahead of the half-1 relu/z evicts
                        # and delay the last z doorbells by a full gather.
                        with tc.tile_wait_until(0.35):
                            nc.scalar.dma_start(
                                zc_sb[:],
                                z_all[c][k * QT * P:(k + 1) * QT * P, :]
                                .rearrange("(t p) o -> p t o", p=P))
                        mt = 8 * k + QT * c
                        a2 = afull_p.tile([P, QT, ROWS], fp8, tag="afull",
                                          name=f"af{c}_{k}")
                        nc.sync.dma_start(
                            a2[:],
                            adjT_e[mt * P:(mt + QT) * P, :]
                            .rearrange("(two p) r -> p two r", p=P))
                        final_grp = (c == ZQ - 1 and k == NCORES - 1)
                        # last group ob-major so ob=0's eviction overlaps
                        # ob=1's final matmuls
                        for ob in range(OB):
                            for ih in range(IH):
                                nc.tensor.matmul(
                                    psum_o[ob][ih][:],
                                    zc_sb[:, 0:QT, ob * P:(ob + 1) * P],
                                    a2[:, 0:QT, ih * HF:(ih + 1) * HF],
                                    start=first, stop=final_grp,
                                    perf_mode=DR,
                                )
                        first = False
                # fused BN affine on PSUM evict: out = psum*scale + bias;
                # out DMAs split across both HWDGE rings
                for ob in range(OB):
                    for ih in range(IH):
                        nc.vector.tensor_scalar(
                            outT_sb[:, ob, ih * HF:(ih + 1) * HF],
                            psum_o[ob][ih][:],
                            bnsc_sb[:, ob:ob + 1],
                            bnbi_sb[:, ob:ob + 1],
                            mybir.AluOpType.mult,
                            mybir.AluOpType.add)
                        eng = nc.sync if ih == 0 else nc.scalar
                        eng.dma_start(
                            out_e[ob * P:(ob + 1) * P,
                                  ih * HF:(ih + 1) * HF],
                            outT_sb[:, ob, ih * HF:(ih + 1) * HF])

    nc.compile()
    return nc


def _get_nc():
    if "nc" not in _cache:
        _cache["nc"] = _build()
    return _cache["nc"]


def kernel(x, IFadj, adj, W1, b1, W2, b2, bn_gamma, bn_beta, bn_mean, bn_var):
    from concourse.bass_utils import run_bass_kernel_spmd

    x = np.asarray(x, dtype=np.float32)
    IFadj = np.asarray(IFadj, dtype=np.float32)
    adj = np.asarray(adj, dtype=np.float32)
    W1 = np.asarray(W1, dtype=np.float32)
    b1 = np.asarray(b1, dtype=np.float32)
    W2 = np.asarray(W2, dtype=np.float32)
    b2 = np.asarray(b2, dtype=np.float32)
    bn_gamma = np.asarray(bn_gamma, dtype=np.float32)
    bn_beta = np.asarray(bn_beta, dtype=np.float32)
    bn_mean = np.asarray(bn_mean, dtype=np.float32)
    bn_var = np.asarray(bn_var, dtype=np.float32)

    # host-side prep: shard rows, transpose for PE lhsT layout, cast.
    # IFadj is mean-shifted before the fp8 cast; the exact rank-1
    # correction 0.5*colsum(x@W1) = 0.5*colsum(x)@W1 goes into the relu
    # bias. W1 is pre-scaled 8x into the e4m3 normal range (the PSUM
    # evict scales by 1/8); W2 is halved so |Z| stays well below the TRN
    # e4m3 max (240); the BN scale is doubled to compensate.
    w1b = (8.0 * W1).astype(_FP8)
    w2b = (0.5 * W2).astype(_BF16)
    b1_eff = b1 + 0.5 * (x.sum(axis=0, dtype=np.float64) @
                         W1.astype(np.float64)).astype(np.float32)
    b1p = np.ascontiguousarray(b1_eff.reshape(JB, P).T)  # [P, JB]
    inv = bn_gamma / np.sqrt(bn_var + BN_EPS)
    bias_tot = b2 * inv + bn_beta - bn_mean * inv
    bnsc = np.ascontiguousarray((2.0 * inv).reshape(OB, P).T)  # [P, OB]
    bnbi = np.ascontiguousarray(bias_tot.reshape(OB, P).T)     # [P, OB]

    # replicated x blocks: m-tiles {8k+GP .. 8k+7} for every k, r-major
    xTr = np.ascontiguousarray(np.concatenate(
        [x[k * ROWS + GP * P:(k + 1) * ROWS] for k in range(NCORES)]
    ).T).astype(_FP8)
    B = IFadj - 0.5  # zero-mean shift: 4x smaller fp8 quantization power

    in_maps = []
    for k in range(NCORES):
        r0, r1 = k * ROWS, (k + 1) * ROWS
        in_maps.append({
            "xTo": np.ascontiguousarray(
                x[r0:r0 + GP * P].T).astype(_FP8),
            "xTr": xTr,
            "ifadjT": np.ascontiguousarray(B[r0:r1].T).astype(_FP8),
            "adjT": np.ascontiguousarray(adj[r0:r1].T).astype(_FP8),
            "w1": w1b,
            "w2": w2b,
            "b1p": b1p,
            "bnsc": bnsc,
            "bnbi": bnbi,
        })

    global _last_in_maps
    _last_in_maps = in_maps

    nc = _get_nc()
    try:
        res = run_bass_kernel_spmd(nc, in_maps, list(range(NCORES)))
    except Exception:
        # transient device wedge (NRT_EXEC_UNIT_UNRECOVERABLE etc.) --
        # a straight retry has been observed to recover
        import time
        time.sleep(2.0)
        res = run_bass_kernel_spmd(nc, in_maps, list(range(NCORES)))
    # per-core output is outT [OUTD, ROWS]; transpose back and stack rows
    return np.concatenate(
        [np.ascontiguousarray(res.results[k]["out"].T)
         for k in range(NCORES)], axis=0)


# revision 13
# speedup vs baseline: 1.1726x; 1.0578x over previous
"""Trainium2 Bass kernel for a 2-layer DGCN (graph conv) on 8 NeuronCores.

Reference computation (fp32):
    h1  = relu(IFadj @ (x @ W1) + b1)         # [N, NHID]
    out = BN(adj @ (h1 @ W2) + b2)            # [N, OUTD], BN in eval mode

Distribution: rows of IFadj / adj (= output rows) are sharded across 8
cores; weights are replicated. S = x @ W1 is split hybrid-style:

  - each core computes its own first GP=2 i-blocks of S and AllGathers
    them (one 512KB gather per block, doorbells at ~5us so the gathers
    run the moment the NRT CC-stream bootstrap window ends -- measured
    ~80-105us in with high run-to-run variance; the first collective
    cannot execute earlier than that no matter when it is issued);
  - the remaining NREP=6 i-blocks of EVERY core-group (48 blocks) are
    computed redundantly on every core (fp8 DoubleRow). Together with
    the REPLICATED part of layer 1 for BOTH row-halves (see below) this
    fills the whole bootstrap window with useful work.

Ordering is the key change vs the naive two-pass layer 1: ALL
gather-independent layer-1 accumulation (48 of 64 m-tiles, both
512-row halves, 8 PSUM banks held open simultaneously) runs BEFORE any
gather-dependent work. The gather-dependent tail per half is only 32
DoubleRow matmuls, so the z doorbells fire ~25us earlier than a
half-at-a-time schedule, and a late S gather never stalls the PE.

z is gathered in 4 quarter-chunks (256 rows, 512KB gather output each,
small enough for the low-latency mesh collective path) pipelined with
phase 4, which consumes m-tiles quarter-major. Inter-core start skew
(measured ~14us, core 0 earliest; core 0's span is the graded time)
gates each gather on the slowest core's doorbell, so doorbell-critical
DMAs (S bounce, z bounces, gather staging) issue from the scalar
engine's HWDGE ring, which is empty -- on the sync ring they would
queue behind ~15MB of input streaming (the baseline's S doorbell fired
at ~41us local because of exactly that).

Per core k (rows R_k):
    phase 1: S-own (2 blocks) -> bounce -> 2 AllGathers; S-rep (48)
    phase 2: h1T = relu(S^T @ BshT_k + b1eff)  (h1 transposed, [NHID, ROWS])
             replicated m-tiles for both halves first; then per half:
             gathered m-tiles, relu-evict, z quarter matmuls, z-quarter
             AllGather doorbells (z PSUM reuses the just-evicted banks).
    phase 3: z_k = h1T.T @ (W2/2), folded into phase 2 per half.
    phase 4: outT = Z-as-lhsT vs adjT_k rhs -> [OUTD, ROWS], fused BN in
             the PSUM-evict op; the host transposes per-core outputs.
             Quarter-chunk-major traversal, one DR pair per (chunk, k).

fp8 scheme (rel-err budget 2e-2; measured ~2.2e-3, same as all-bf16):
  All three big matmuls run in fp8e4m3 with perf_mode=DoubleRow (two
  128-row contraction tiles per matmul, half the DMA/collective bytes).
  Naive e4m3 on uniform[0,1) adjacency loses ~1.7e-2 rel err via a
  per-column bias in h1 that the next adjacency matmul amplifies
  ~4096x; instead IFadj is mean-shifted on the host (B = IFadj - 0.5)
  and the exact rank-1 correction 0.5*colsum(x @ W1) is folded into the
  relu bias -- computed exactly on the host (this exactness matters:
  the correction also cancels the column-bias of quantizing S). W1 is
  pre-scaled 8x into the e4m3 normal range (the PSUM evict undoes it),
  and W2 by 0.5 so |Z|<120 stays far from the TRN e4m3 max of 240 (the
  BN scale is doubled to undo that). h1 stays bf16 and W2 bf16:
  quantizing W2 puts a column-bias on Z that L2's adjacency matmul
  amplifies.

The PE consumes the left operand transposed (out = lhsT.T @ rhs), so the
host passes IFadj[R_k].T / adj[R_k].T / x-slices.T per core; with the
h1T / outT formulations no on-device transposes are needed anywhere.
Accumulation everywhere is fp32.
"""

import numpy as np
import ml_dtypes

NCORES = 8
N = 8192
NFEAT = 1024
NHID = 512
OUTD = 256
ROWS = N // NCORES  # 1024
P = 128
BN_EPS = 1e-5

CB = NFEAT // P   # 8  c-blocks (x feature contraction)
IB = ROWS // P    # 8  i-blocks (local rows)
JB = NHID // P    # 4  j-blocks (hidden)
MT = N // P       # 64 m-tiles (global node contraction)
HF = 512          # matmul moving free dim (PSUM bank limit)
IH = ROWS // HF   # 2 i-halves of the local row range
OB = OUTD // P    # 2 output-feature blocks
GP = 2            # i-blocks per core gathered (the rest replicated)
NREP = IB - GP    # i-blocks per core-group replicated on every core (6)
ZG = 2            # z allgathers (one per row-half, 4 i-blocks each)

_BF16 = ml_dtypes.bfloat16
_FP8 = ml_dtypes.float8_e4m3  # TRN fp8e4 (IEEE-style, max 240)

_cache = {}


def _build():
    import concourse.mybir as mybir
    import concourse.tile as tile
    from concourse import bacc

    dt = mybir.dt
    f32 = dt.float32
    bf16 = dt.bfloat16
    fp8 = dt.float8e4
    AF = mybir.ActivationFunctionType
    DR = mybir.MatmulPerfMode.DoubleRow

    nc = bacc.Bacc("TRN2", target_bir_lowering=False, debug=False,
                   num_devices=NCORES)

    # xTo: core's own first GP i-blocks (gathered); xTr: the NREP-per-group
    # replicated blocks, identical on every core, packed r = NREP*k + (j-GP)
    xTo_e = nc.dram_tensor("xTo", [NFEAT, GP * P], fp8, kind="ExternalInput")
    xTr_e = nc.dram_tensor("xTr", [NFEAT, NREP * NCORES * P], fp8,
                           kind="ExternalInput")
    ifadjT_e = nc.dram_tensor("ifadjT", [N, ROWS], fp8, kind="ExternalInput")
    adjT_e = nc.dram_tensor("adjT", [N, ROWS], fp8, kind="ExternalInput")
    w1_e = nc.dram_tensor("w1", [NFEAT, NHID], fp8, kind="ExternalInput")
    w2_e = nc.dram_tensor("w2", [NHID, OUTD], bf16, kind="ExternalInput")
    b1p_e = nc.dram_tensor("b1p", [P, JB], f32, kind="ExternalInput")
    bnsc_e = nc.dram_tensor("bnsc", [P, OB], f32, kind="ExternalInput")
    bnbi_e = nc.dram_tensor("bnbi", [P, OB], f32, kind="ExternalInput")
    # outT: [OUTD, ROWS]; the host transposes each core's block.
    out_e = nc.dram_tensor("out", [OUTD, ROWS], f32, kind="ExternalOutput")

    groups = [list(range(NCORES))]

    def allgather(g_in, g_out):
        nc.gpsimd.collective_compute(
            "AllGather", mybir.AluOpType.bypass, replica_groups=groups,
            ins=[g_in[:]], outs=[g_out[:]])

    with tile.TileContext(nc) as tc:
        with (
            tc.tile_pool(name="const", bufs=1) as const,
            tc.tile_pool(name="srep", bufs=1) as srep_p,
            tc.tile_pool(name="schunk", bufs=NCORES) as schunk_p,
            tc.tile_pool(name="h1", bufs=1) as h1_p,
            tc.tile_pool(name="zsb", bufs=1) as z_p,
            tc.tile_pool(name="zchunk", bufs=8) as zchunk_p,
            tc.tile_pool(name="arep", bufs=3) as arep_p,
            tc.tile_pool(name="agat", bufs=NCORES) as agat_p,
            tc.tile_pool(name="afull", bufs=8) as afull_p,
            tc.tile_pool(name="outsb", bufs=1) as outsb_p,
            tc.tile_pool(name="dram", bufs=1, space="DRAM") as dram,
        ):
            # ---- constants into SBUF, in consumption order: w1 and the
            # core's own x-blocks first (S-own + its gather doorbells go
            # out ~5us into the kernel), then the replicated x stream.
            w1_sb = const.tile([P, CB, NHID], fp8)
            w1_r = w1_e[:].rearrange("(cb p) j -> p cb j", p=P)
            # first cb-pair separately: the first S matmul fires ~1us sooner
            nc.sync.dma_start(w1_sb[:, 0:2, :], w1_r[:, 0:2, :])
            nc.sync.dma_start(w1_sb[:, 2:CB, :], w1_r[:, 2:CB, :])
            xTo_sb = const.tile([P, CB, GP * P], fp8)
            nc.sync.dma_start(
                xTo_sb[:], xTo_e[:].rearrange("(cb p) i -> p cb i", p=P))
            NRC = NREP * NCORES * P  # replicated x columns (6144)
            xTr_sb = const.tile([P, CB, NRC], fp8)
            xTr_r = xTr_e[:].rearrange("(cb p) i -> p cb i", p=P)
            MG = 8  # replicated-x DMA granularity (~0.75MB per transfer)
            nc.sync.dma_start(xTr_sb[:, :, 0:NRC // MG],
                              xTr_r[:, :, 0:NRC // MG])
            # late-consumed consts slot in behind xTr chunk 0 (the first
            # S-rep blocks chew on chunk 0 for ~6us before needing chunk 1)
            w2_sb = const.tile([P, JB, OUTD], bf16)
            nc.sync.dma_start(
                w2_sb[:], w2_e[:].rearrange("(jb p) o -> p jb o", p=P))
            b1p_sb = const.tile([P, JB], f32)
            nc.sync.dma_start(b1p_sb[:], b1p_e[:])
            bnsc_sb = const.tile([P, OB], f32)
            nc.sync.dma_start(bnsc_sb[:], bnsc_e[:])
            bnbi_sb = const.tile([P, OB], f32)
            nc.sync.dma_start(bnbi_sb[:], bnbi_e[:])
            for g in range(1, MG):
                nc.sync.dma_start(
                    xTr_sb[:, :, g * (NRC // MG):(g + 1) * (NRC // MG)],
                    xTr_r[:, :, g * (NRC // MG):(g + 1) * (NRC // MG)])

            # ---- DRAM bounce buffers for the collectives. Collective
            # count matters: each op costs ~10-15us of serial CC-stream
            # time regardless of size (mesh 512KB and RDH 1MB measure
            # about the same), so use the minimum: one S gather + one z
            # gather per row-half.
            s_bounce = dram.tile([GP * P, NHID], fp8, name="sb")
            s_all = dram.tile([GP * P * NCORES, NHID], fp8,
                              addr_space="Shared", name="sa")
            RPC = (IB // ZG) * P  # rows bounced per z gather (512)
            z_bounce = [dram.tile([RPC, OUTD], fp8, name=f"zb{q}")
                        for q in range(ZG)]
            z_all = [dram.tile([RPC * NCORES, OUTD], fp8,
                               addr_space="Shared", name=f"za{q}")
                     for q in range(ZG)]

            # ---- phase 1: S = x @ (8*W1) / 8, fp8 DoubleRow over cb-pairs
            # (256-feature contraction per matmul). Own blocks first ->
            # bounce (scalar ring: empty, so the gather doorbells fire at
            # ~5us) -> one gather per block; then the 48 replicated blocks.
            s_own = srep_p.tile([P, GP, NHID], fp8)
            s_rep = srep_p.tile([P, NREP * NCORES, NHID], fp8)

            def s_block(ps1, x_sb, ib_src, s_dst):
                ps = ps1.tile([P, NHID], f32, tag="s")
                for cp in range(CB // 2):
                    cb0 = 2 * cp
                    nc.tensor.matmul(
                        ps[:],
                        x_sb[:, cb0:cb0 + 2, ib_src * P:(ib_src + 1) * P],
                        w1_sb[:, cb0:cb0 + 2, :],
                        start=(cp == 0), stop=(cp == CB // 2 - 1),
                        perf_mode=DR,
                    )
                nc.scalar.activation(s_dst, ps[:], AF.Copy, scale=0.125)

            with tc.tile_pool(name="ps1", bufs=2, space="PSUM") as ps1:
                for ib in range(GP):
                    s_block(ps1, xTo_sb, ib, s_own[:, ib, :])
                    nc.scalar.dma_start(
                        s_bounce[ib * P:(ib + 1) * P, :], s_own[:, ib, :])
                allgather(s_bounce, s_all)
                for r in range(NREP * NCORES):
                    s_block(ps1, xTr_sb, r, s_rep[:, r, :])

            h1T = h1_p.tile([P, JB, ROWS], bf16)
            z_sb = z_p.tile([P, IB, OUTD], fp8)

            # gathered-part adjacency tiles: small (2MB total), prefetched
            # before the replicated stream so they are resident when the
            # gather lands (the replicated a6 stream is pool-gated and
            # would head-of-line-block them on the sync ring otherwise).
            a2g = []
            for k in range(NCORES):
                a2 = agat_p.tile([P, GP, ROWS], fp8, tag="a2g",
                                 name=f"a2g{k}")
                nc.sync.dma_start(
                    a2[:],
                    ifadjT_e[8 * k * P:(8 * k + GP) * P, :]
                    .rearrange("(two p) f -> p two f", p=P))
                a2g.append(a2)

            # ---- phase 2+3: all 8 half x j-block PSUM accumulators open
            # at once (exactly the 8 banks). Replicated m-tiles of BOTH
            # halves first; then per half: gathered m-tiles, relu-evict,
            # z quarters + their gather doorbells (z PSUM reuses the
            # just-evicted banks via tag-slot rotation).
            with tc.tile_pool(name="ps2", bufs=1, space="PSUM") as ps2:
                psum_h = [[ps2.tile([P, HF], f32, name=f"ph{jb}_{ih}",
                                    tag=f"ph{jb}_{ih}")
                           for ih in range(IH)] for jb in range(JB)]
                started = set()

                def dr_pair(s_sl, a_sl, jb, ih, stop):
                    nc.tensor.matmul(
                        psum_h[jb][ih][:], s_sl, a_sl,
                        start=((jb, ih) not in started), stop=stop,
                        perf_mode=DR,
                    )
                    started.add((jb, ih))

                # replicated part: per core-group k, m-tiles 8k+2 .. 8k+7
                # as one full-width 6-tile DMA (0.75MB)
                for k in range(NCORES):
                    r0 = NREP * k  # s_rep index of m-tile 8k+GP
                    mt = 8 * k + GP
                    a6 = arep_p.tile([P, NREP, ROWS], fp8, tag="a6",
                                     name=f"a6_{k}")
                    nc.sync.dma_start(
                        a6[:],
                        ifadjT_e[mt * P:(mt + NREP) * P, :]
                        .rearrange("(six p) f -> p six f", p=P))
                    for pp in range(NREP // 2):
                        for jb in range(JB):
                            for ih in range(IH):
                                dr_pair(
                                    s_rep[:, r0 + 2 * pp:r0 + 2 * pp + 2,
                                          jb * P:(jb + 1) * P],
                                    a6[:, 2 * pp:2 * pp + 2,
                                       ih * HF:(ih + 1) * HF],
                                    jb, ih, stop=False)

                # gathered-S staging: core-block k -> GP m-tiles {8k,8k+1};
                # staged from the scalar ring so it fires the moment each
                # gather lands. Issued AFTER the replicated sweep: these
                # triggers run on the Scalar ENGINE's queue and wait on
                # the gather sems -- issued any earlier they head-of-line
                # block the phase-1 PSUM-evict ACTIVATEs behind them and
                # deadlock the S pipeline until the gather lands.
                # tile_wait_until: the scheduler's cost model thinks the
                # gathers complete in a few us, so without the override it
                # statically interleaves these gather-waiting triggers into
                # the Scalar ENGINE queue ahead of still-pending phase-1
                # evict ACTIVATEs -- the trigger then head-of-line blocks
                # the ACT queue for the real ~50-90us bootstrap and stalls
                # the whole S pipeline behind it.
                s_gat = []
                with tc.tile_wait_until(0.25):
                    for k in range(NCORES):
                        sg = schunk_p.tile([P, GP, NHID], fp8, tag="schunk",
                                           name=f"sg{k}")
                        nc.scalar.dma_start(
                            sg[:],
                            s_all[k * GP * P:(k + 1) * GP * P, :]
                            .rearrange("(t p) j -> p t j", p=P))
                        s_gat.append(sg)

                # per half: gathered m-tiles {8k, 8k+1}, evict, z, gathers
                for ih in range(IH):
                    for k in range(NCORES):
                        for jb in range(JB):
                            dr_pair(
                                s_gat[k][:, :, jb * P:(jb + 1) * P],
                                a2g[k][:, :, ih * HF:(ih + 1) * HF],
                                jb, ih, stop=(k == NCORES - 1))
                    for jb in range(JB):
                        nc.scalar.activation(
                            h1T[:, jb, ih * HF:(ih + 1) * HF],
                            psum_h[jb][ih][:], AF.Relu,
                            bias=b1p_sb[:, jb:jb + 1])
                    # z for this half's i-blocks, bounce, gather the half
                    for t in range(IB // IH):
                        ib = (IB // IH) * ih + t
                        ps = ps2.tile([P, OUTD], f32, name=f"z{ib}",
                                      tag=f"ph{t}_{ih}")
                        for jb in range(JB):
                            nc.tensor.matmul(
                                ps[:],
                                h1T[:, jb, ib * P:(ib + 1) * P],
                                w2_sb[:, jb, :],
                                start=(jb == 0), stop=(jb == JB - 1),
                            )
                        nc.scalar.activation(z_sb[:, ib, :], ps[:],
                                             AF.Copy)
                        nc.scalar.dma_start(
                            z_bounce[ih][t * P:(t + 1) * P, :],
                            z_sb[:, ib, :])
                    allgather(z_bounce[ih], z_all[ih])

            # ---- phase 4: outT[o, i] = sum_m Z[m, o] * adjT[m, i], BN
            # fused. z-chunk-major traversal: chunk c holds m-tiles
            # {8k + 4c + q}. DR pairs of m-tiles, fp8 both operands.
            # z staging on the scalar ring.
            outT_sb = outsb_p.tile([P, OB, ROWS], f32)
            QT = IB // ZG  # m-tiles per (chunk, group) = 4
            with tc.tile_pool(name="ps4", bufs=1, space="PSUM") as ps4:
                psum_o = [[ps4.tile([P, HF], f32, name=f"po{ob}_{ih}",
                                    tag=f"po{ob}_{ih}")
                           for ih in range(IH)] for ob in range(OB)]
                first = True
                for c in range(ZG):
                    for k in range(NCORES):
                        zc_sb = zchunk_p.tile([P, QT, OUTD], fp8,
                                              tag="zchunk", name=f"zc{c}_{k}")
                        # wait override: same scheduler hazard as stage_s --
                        # unhinted, these z-gather-waiting triggers land in
                        # the ACT queue ahead of the half-1 relu/z evicts
                        # and delay the last z doorbells by a full gather.
                        with tc.tile_wait_until(0.35):
                            nc.scalar.dma_start(
                                zc_sb[:],
                                z_all[c][k * QT * P:(k + 1) * QT * P, :]
                                .rearrange("(t p) o -> p t o", p=P))
                        mt = 8 * k + QT * c
                        a4 = afull_p.tile([P, QT, ROWS], fp8, tag="afull",
                                          name=f"af{c}_{k}")
                        nc.sync.dma_start(
                            a4[:],
                            adjT_e[mt * P:(mt + QT) * P, :]
                            .rearrange("(four p) r -> p four r", p=P))
                        final_grp = (c == ZG - 1 and k == NCORES - 1)
                        # last group ob-major so ob=0's eviction overlaps
                        # ob=1's final matmuls
                        for ob in range(OB):
                            for ih in range(IH):
                                for qp in range(QT // 2):
                                    q0 = 2 * qp
                                    nc.tensor.matmul(
                                        psum_o[ob][ih][:],
                                        zc_sb[:, q0:q0 + 2,
                                              ob * P:(ob + 1) * P],
                                        a4[:, q0:q0 + 2,
                                           ih * HF:(ih + 1) * HF],
                                        start=(first and qp == 0),
                                        stop=(final_grp and qp == QT // 2 - 1),
                                        perf_mode=DR,
                                    )
                        first = False
                # fused BN affine on PSUM evict: out = psum*scale + bias;
                # out DMAs split across both HWDGE rings
                for ob in range(OB):
                    for ih in range(IH):
                        nc.vector.tensor_scalar(
                            outT_sb[:, ob, ih * HF:(ih + 1) * HF],
                            psum_o[ob][ih][:],
                            bnsc_sb[:, ob:ob + 1],
                            bnbi_sb[:, ob:ob + 1],
                            mybir.AluOpType.mult,
                            mybir.AluOpType.add)
                        eng = nc.sync if ih == 0 else nc.scalar
                        eng.dma_start(
                            out_e[ob * P:(ob + 1) * P,
                                  ih * HF:(ih + 1) * HF],
                            outT_sb[:, ob, ih * HF:(ih + 1) * HF])

    nc.compile()
    return nc


def _get_nc():
    if "nc" not in _cache:
        _cache["nc"] = _build()
    return _cache["nc"]


def kernel(x, IFadj, adj, W1, b1, W2, b2, bn_gamma, bn_beta, bn_mean, bn_var):
    from concourse.bass_utils import run_bass_kernel_spmd

    x = np.asarray(x, dtype=np.float32)
    IFadj = np.asarray(IFadj, dtype=np.float32)
    adj = np.asarray(adj, dtype=np.float32)
    W1 = np.asarray(W1, dtype=np.float32)
    b1 = np.asarray(b1, dtype=np.float32)
    W2 = np.asarray(W2, dtype=np.float32)
    b2 = np.asarray(b2, dtype=np.float32)
    bn_gamma = np.asarray(bn_gamma, dtype=np.float32)
    bn_beta = np.asarray(bn_beta, dtype=np.float32)
    bn_mean = np.asarray(bn_mean, dtype=np.float32)
    bn_var = np.asarray(bn_var, dtype=np.float32)

    # host-side prep: shard rows, transpose for PE lhsT layout, cast.
    # IFadj is mean-shifted before the fp8 cast; the exact rank-1
    # correction 0.5*colsum(x@W1) = 0.5*colsum(x)@W1 goes into the relu
    # bias. W1 is pre-scaled 8x into the e4m3 normal range (the PSUM
    # evict scales by 1/8); W2 is halved so |Z| stays well below the TRN
    # e4m3 max (240); the BN scale is doubled to compensate.
    w1b = (8.0 * W1).astype(_FP8)
    w2b = (0.5 * W2).astype(_BF16)
    b1_eff = b1 + 0.5 * (x.sum(axis=0, dtype=np.float64) @
                         W1.astype(np.float64)).astype(np.float32)
    b1p = np.ascontiguousarray(b1_eff.reshape(JB, P).T)  # [P, JB]
    inv = bn_gamma / np.sqrt(bn_var + BN_EPS)
    bias_tot = b2 * inv + bn_beta - bn_mean * inv
    bnsc = np.ascontiguousarray((2.0 * inv).reshape(OB, P).T)  # [P, OB]
    bnbi = np.ascontiguousarray(bias_tot.reshape(OB, P).T)     # [P, OB]

    # replicated x blocks: m-tiles {8k+GP .. 8k+7} for every k, r-major
    xTr = np.ascontiguousarray(np.concatenate(
        [x[k * ROWS + GP * P:(k + 1) * ROWS] for k in range(NCORES)]
    ).T).astype(_FP8)
    B = IFadj - 0.5  # zero-mean shift: 4x smaller fp8 quantization power

    in_maps = []
    for k in range(NCORES):
        r0, r1 = k * ROWS, (k + 1) * ROWS
        in_maps.append({
            "xTo": np.ascontiguousarray(
                x[r0:r0 + GP * P].T).astype(_FP8),
            "xTr": xTr,
            "ifadjT": np.ascontiguousarray(B[r0:r1].T).astype(_FP8),
            "adjT": np.ascontiguousarray(adj[r0:r1].T).astype(_FP8),
            "w1": w1b,
            "w2": w2b,
            "b1p": b1p,
            "bnsc": bnsc,
            "bnbi": bnbi,
        })

    global _last_in_maps
    _last_in_maps = in_maps

    nc = _get_nc()
    try:
        res = run_bass_kernel_spmd(nc, in_maps, list(range(NCORES)))
    except Exception:
        # transient device wedge (NRT_EXEC_UNIT_UNRECOVERABLE etc.) --
        # a straight retry has been observed to recover
        import time
        time.sleep(2.0)
        res = run_bass_kernel_spmd(nc, in_maps, list(range(NCORES)))
    # per-core output is outT [OUTD, ROWS]; transpose back and stack rows
    return np.concatenate(
        [np.ascontiguousarray(res.results[k]["out"].T)
         for k in range(NCORES)], axis=0)
